# revision 11
# baseline (speedup 1.0000x reference)
"""GQA attention (RoPE + causal softmax + out-proj) on 8 Trainium2 cores.

Sharding: DP=2 over batch x TP=4 over KV groups.
  core c: batch b=c//4, tp=c%4 -> local q heads [8*tp, 8*tp+8), kv heads
  [2*tp, 2*tp+2).
Stages (per core):
  1. x^T via PE transposes (SBUF-resident chunk), QKV projection (fp32r
     matmuls), RoPE on DVE -> q^T/k^T/v to DRAM scratch.
  2. Flash-style causal attention per head in S^T orientation
     (scores^T = k^T_tile^T . q^T), exp on ACT, denominator via
     ones-matmul, PV accumulation in PSUM -> attnout^T [1024, S] DRAM.
  3. AllGather attnout^T within each 4-core group -> [4096, S].
  4. Out-projection with wo sharded over OUTPUT dim -> y [S, 1024];
     host concatenates the 4 dout shards per batch.
All matmuls run as float32r (full fp32 storage; PE relaxed-precision mode,
4x faster than strict fp32).
"""
import sys

sys.path.insert(0, "/opt/trn_rl_repo")

import numpy as np

import concourse.bacc as bacc
import concourse.mybir as mybir
from concourse import tile
from concourse.bass_utils import run_bass_kernel_spmd
from concourse.masks import make_identity

F32 = mybir.dt.float32
F32R = mybir.dt.float32r
EXP = mybir.ActivationFunctionType.Exp
MULT = mybir.AluOpType.mult


class Cfg:
    def __init__(self, B=2, S=2048, D=4096, H=32, KV=8, HD=128, TP=4,
                 TCHUNK=1024, NQ=512):
        self.B, self.S, self.D, self.H, self.KV, self.HD = B, S, D, H, KV, HD
        self.TP = TP
        self.DP = B
        self.NCORES = TP * B
        self.G = H // KV                    # q heads per kv head
        self.HL = H // TP                   # local q heads
        self.KVL = KV // TP                 # local kv heads
        self.DT = D // 128                  # contraction d-tiles
        self.QCOLS = self.HL * HD           # 1024
        self.KCOLS = self.KVL * HD          # 256
        self.WCOLS = self.QCOLS + 2 * self.KCOLS
        self.TCHUNK = TCHUNK
        self.NCH = S // TCHUNK
        self.TH = TCHUNK // 512             # 512-wide t-halves per chunk
        self.NQ = NQ                        # attention qt-chunk (<=512)
        self.NMASK = NQ // 128
        self.NQC = S // NQ
        self.OUTC = self.D // TP            # local wo out-cols (1024)
        self.AT = (H * HD) // 128           # attnout hd-tiles
        assert HD == 128 and TCHUNK % 512 == 0 and NQ % 128 == 0


def _make_sets(cfg):
    """Partition stage-1 QKV psum groups into sets of <=6 (psum banks),
    each with the wqkvT column range it needs."""
    groups = []
    for hd in range(cfg.HL):
        for th in range(cfg.TH):
            lo = hd * 128
            groups.append(("q", hd, th, lo, lo + 128))
    for kh in range(cfg.KVL):
        for th in range(cfg.TH):
            lo = cfg.QCOLS + kh * 128
            groups.append(("k", kh, th, lo, lo + 128))
    vlo = cfg.QCOLS + cfg.KCOLS
    for ts in range(cfg.TCHUNK // 128):
        groups.append(("v", ts, 0, vlo, vlo + cfg.KCOLS))
    sets = []
    for i in range(0, len(groups), 6):
        chunk = groups[i:i + 6]
        lo = min(g[3] for g in chunk)
        hi = max(g[4] for g in chunk)
        sets.append((lo, hi, chunk))
    return sets


def build_program(cfg, num_devices, replica_groups, fake_ag=False):
    nc = bacc.Bacc(trn_type="TRN2", target_bir_lowering=False, debug=False,
                   num_devices=num_devices)
    S, D, HD = cfg.S, cfg.D, cfg.HD
    scale = float(1.0 / np.sqrt(HD))

    x_d = nc.dram_tensor("x", [S, D], F32, kind="ExternalInput").ap()
    w_d = nc.dram_tensor("wqkvT", [D, cfg.WCOLS], F32R, kind="ExternalInput").ap()
    wo_d = nc.dram_tensor("woT", [cfg.H * HD, cfg.OUTC], F32R,
                          kind="ExternalInput").ap()
    p1_d = nc.dram_tensor("p1", [128, S], F32, kind="ExternalInput").ap()
    p2_d = nc.dram_tensor("p2", [128, S], F32, kind="ExternalInput").ap()
    mk_d = nc.dram_tensor("masks", [128, cfg.NMASK, cfg.NQ], F32R,
                          kind="ExternalInput").ap()
    o1_d = nc.dram_tensor("ones1", [128, 1], F32R, kind="ExternalInput").ap()
    o2_d = nc.dram_tensor("ones2", [1, 128], F32R, kind="ExternalInput").ap()
    y_d = nc.dram_tensor("y", [S, cfg.OUTC], F32, kind="ExternalOutput").ap()

    sets = _make_sets(cfg)

    with tile.TileContext(nc) as tc, nc.allow_low_precision(reason="fp32r keeps 32-bit storage; PE relaxed mode"):
        with (
            tc.tile_pool(name="dram", bufs=1, space="DRAM") as pd,
            tc.tile_pool(name="const", bufs=1) as pc,
        ):
            qT_d = pd.tile([cfg.QCOLS, S], F32R)
            kT_d = pd.tile([cfg.KCOLS, S], F32R)
            v_d = pd.tile([S, cfg.KCOLS], F32R)
            ao_d = pd.tile([cfg.QCOLS, S], F32R)
            ag_d = pd.tile([cfg.QCOLS * cfg.TP, S], F32R)

            ones = pc.tile([128, 1], F32R)
            nc.sync.dma_start(ones[:], o1_d)
            ones_r = pc.tile([1, 128], F32R)
            nc.sync.dma_start(ones_r[:], o2_d)

            # ---------------- stage 1: x^T + QKV + RoPE ----------------
            with (
                tc.tile_pool(name="s1xT", bufs=1) as pxT,
                tc.tile_pool(name="s1xn", bufs=2) as pxn,
                tc.tile_pool(name="s1w", bufs=2) as pw,
                tc.tile_pool(name="s1t", bufs=2) as pt,
                tc.tile_pool(name="s1o", bufs=3) as po,
                tc.tile_pool(name="s1c", bufs=1) as pc1,
                tc.tile_pool(name="s1ps", bufs=8, space="PSUM") as pps,
            ):
                ident = pc1.tile([128, 128], F32)
                make_identity(nc, ident[:])
                for c0 in range(cfg.NCH):
                    t0 = c0 * cfg.TCHUNK
                    p1s = pc1.tile([128, cfg.TCHUNK], F32, tag="p1s", bufs=2)
                    p2s = pc1.tile([128, cfg.TCHUNK], F32, tag="p2s", bufs=2)
                    nc.sync.dma_start(p1s[:], p1_d[:, t0:t0 + cfg.TCHUNK])
                    nc.sync.dma_start(p2s[:], p2_d[:, t0:t0 + cfg.TCHUNK])

                    # transpose x[t0:t0+TCHUNK, :] -> xT tiles [128, DT, TCHUNK]
                    xT = pxT.tile([128, cfg.DT, cfg.TCHUNK], F32R, tag="xT")
                    nts = cfg.TCHUNK // 128
                    for db in range(cfg.DT // 2):          # d-blocks of 256
                        xn = pxn.tile([128, nts, 256], F32, tag="xn")
                        nc.sync.dma_start(
                            xn[:],
                            x_d[t0:t0 + cfg.TCHUNK, db * 256:(db + 1) * 256]
                            .rearrange("(ts p) d -> p ts d", p=128))
                        for ds in range(2):
                            dt = db * 2 + ds
                            for th in range(nts // 4):     # 4 transposes/bank
                                pst = pps.tile([128, 512], F32, tag="ps")
                                for q4 in range(4):
                                    ts = th * 4 + q4
                                    nc.tensor.transpose(
                                        pst[:, q4 * 128:(q4 + 1) * 128],
                                        xn[:, ts, ds * 128:(ds + 1) * 128],
                                        ident[:])
                                nc.vector.tensor_copy(
                                    xT[:, dt, th * 512:(th + 1) * 512], pst[:])

                    # QKV by column-sets
                    for (lo, hi, grps) in sets:
                        psums = []
                        for g in grps:
                            ps = pps.tile([128, 512], F32, tag="ps")
                            psums.append(ps)
                        for dt in range(cfg.DT):
                            wt = pw.tile([128, cfg.WCOLS], F32R, tag="wt")
                            nc.sync.dma_start(wt[:, 0:hi - lo],
                                              w_d[dt * 128:(dt + 1) * 128, lo:hi])
                            st = (dt == 0)
                            sp = (dt == cfg.DT - 1)
                            for ps, (kind, a, th, glo, ghi) in zip(psums, grps):
                                if kind == "v":
                                    nc.tensor.matmul(
                                        ps[:, 0:cfg.KCOLS],
                                        xT[:, dt, a * 128:(a + 1) * 128],
                                        wt[:, glo - lo:ghi - lo],
                                        start=st, stop=sp)
                                else:
                                    nc.tensor.matmul(
                                        ps[:],
                                        wt[:, glo - lo:ghi - lo],
                                        xT[:, dt, th * 512:(th + 1) * 512],
                                        start=st, stop=sp)
                        # evacuate psums
                        for ps, (kind, a, th, glo, ghi) in zip(psums, grps):
                            if kind == "v":
                                vt = po.tile([128, cfg.KCOLS], F32R, tag="vt")
                                nc.vector.tensor_copy(vt[:], ps[:, 0:cfg.KCOLS])
                                r0 = t0 + a * 128
                                nc.sync.dma_start(v_d[r0:r0 + 128, :], vt[:])
                            else:
                                # RoPE: out = in*P1 + swap(in)*P2
                                c_lo = th * 512
                                c_hi = c_lo + 512
                                t1 = pt.tile([128, 512], F32, tag="t1")
                                t2 = pt.tile([128, 512], F32, tag="t2")
                                ot = po.tile([128, 512], F32R, tag="ot")
                                nc.vector.tensor_mul(
                                    t1[:], ps[:], p1s[:, c_lo:c_hi])
                                nc.vector.tensor_mul(
                                    t2[0:64, :], ps[64:128, :],
                                    p2s[0:64, c_lo:c_hi])
                                nc.vector.tensor_mul(
                                    t2[64:128, :], ps[0:64, :],
                                    p2s[64:128, c_lo:c_hi])
                                nc.vector.tensor_add(ot[:], t1[:], t2[:])
                                dst = qT_d if kind == "q" else kT_d
                                r0 = a * 128
                                nc.sync.dma_start(
                                    dst[r0:r0 + 128, t0 + c_lo:t0 + c_hi],
                                    ot[:])

            # ---------------- stage 2: attention ----------------
            with (
                tc.tile_pool(name="s2kv", bufs=2) as pkv,
                tc.tile_pool(name="s2q", bufs=2) as pq,
                tc.tile_pool(name="s2w", bufs=6) as pw2,
                tc.tile_pool(name="s2c", bufs=1) as pc2,
                tc.tile_pool(name="s2ps", bufs=1, space="PSUM") as pps2,
            ):
                msk = pc2.tile([128, cfg.NMASK, cfg.NQ], F32R)
                nc.sync.dma_start(msk[:], mk_d)
                for kv in range(cfg.KVL):
                    kT_s = pkv.tile([128, S], F32R, tag="kT")
                    v_s = pkv.tile([128, S // 128, 128], F32R, tag="v")
                    nc.sync.dma_start(kT_s[:], kT_d[kv * 128:(kv + 1) * 128, :])
                    nc.sync.dma_start(
                        v_s[:],
                        v_d[:, kv * 128:(kv + 1) * 128]
                        .rearrange("(tt p) h -> p tt h", p=128))
                    for qh in range(cfg.G):
                        h = kv * cfg.G + qh
                        q_s = pq.tile([128, S], F32R, tag="q")
                        nc.sync.dma_start(q_s[:], qT_d[h * 128:(h + 1) * 128, :])
                        for qc in range(cfg.NQC):
                            nkt = (qc + 1) * (cfg.NQ // 128)
                            cl = qc * cfg.NQ
                            ch = cl + cfg.NQ
                            acc = pw2.tile([128, cfg.NQ], F32R, tag="acc", bufs=2)
                            aop = pps2.tile([128, cfg.NQ], F32, tag="aop", bufs=2)
                            for ki in range(nkt):
                                sps = pps2.tile([128, cfg.NQ], F32, tag="sps", bufs=3)
                                nc.tensor.matmul(
                                    sps[:],
                                    kT_s[:, ki * 128:(ki + 1) * 128],
                                    q_s[:, cl:ch],
                                    start=True, stop=True)
                                et = pw2.tile([128, cfg.NQ], F32R, tag="et")
                                nc.scalar.activation(et[:], sps[:], EXP,
                                                     scale=scale)
                                di = ki - qc * (cfg.NQ // 128)
                                if di >= 0:
                                    mt = pw2.tile([128, cfg.NQ], F32R, tag="mt")
                                    nc.vector.tensor_mul(mt[:], et[:],
                                                         msk[:, di, :])
                                    use = mt
                                else:
                                    use = et
                                nc.tensor.matmul(
                                    aop[:], v_s[:, ki, :],
                                    use[:],
                                    start=(ki == 0), stop=(ki == nkt - 1))
                                if ki == 0:
                                    nc.vector.tensor_copy(acc[:], use[:])
                                else:
                                    nc.vector.tensor_add(acc[:], acc[:], use[:])
                            dps = pps2.tile([1, cfg.NQ], F32, tag="dps", bufs=1)
                            nc.tensor.matmul(dps[:], ones[:],
                                             acc[:],
                                             start=True, stop=True)
                            rec = pw2.tile([1, cfg.NQ], F32R, tag="rec", bufs=2)
                            nc.vector.reciprocal(rec[:], dps[:])
                            rbp = pps2.tile([128, cfg.NQ], F32, tag="rbp", bufs=1)
                            nc.tensor.matmul(rbp[:], ones_r[:],
                                             rec[:],
                                             start=True, stop=True)
                            rbs = pw2.tile([128, cfg.NQ], F32, tag="rbs", bufs=2)
                            nc.vector.tensor_copy(rbs[:], rbp[:])
                            aos = pw2.tile([128, cfg.NQ], F32R, tag="aos")
                            nc.vector.tensor_tensor(aos[:], aop[:], rbs[:],
                                                    op=MULT)
                            nc.sync.dma_start(
                                ao_d[h * 128:(h + 1) * 128, cl:ch], aos[:])

            # ---------------- stage 3: AllGather + out-proj ----------------
            if fake_ag:
                for r in range(cfg.TP):
                    nc.sync.dma_start(
                        ag_d[r * cfg.QCOLS:(r + 1) * cfg.QCOLS, :], ao_d[:])
            else:
                nc.gpsimd.collective_compute(
                    "AllGather", mybir.AluOpType.bypass,
                    replica_groups=replica_groups,
                    ins=[ao_d[:].opt()], outs=[ag_d[:].opt()])

            with (
                tc.tile_pool(name="s3wo", bufs=1) as pwo,
                tc.tile_pool(name="s3a", bufs=3) as pa,
                tc.tile_pool(name="s3y", bufs=4) as py,
                tc.tile_pool(name="s3ps", bufs=4, space="PSUM") as pps3,
            ):
                wo_s = pwo.tile([128, cfg.AT, cfg.OUTC], F32R)
                nc.sync.dma_start(
                    wo_s[:], wo_d.rearrange("(dt p) o -> p dt o", p=128))
                for tt in range(S // 128):
                    at_s = pa.tile([128, cfg.AT, 128], F32R, tag="at")
                    nc.sync.dma_start(
                        at_s[:],
                        ag_d[:, tt * 128:(tt + 1) * 128]
                        .rearrange("(a p) t -> p a t", p=128))
                    ocw = min(512, cfg.OUTC)
                    for oc in range(cfg.OUTC // ocw):
                        yps = pps3.tile([128, ocw], F32, tag="yps")
                        for a in range(cfg.AT):
                            nc.tensor.matmul(
                                yps[:], at_s[:, a, :],
                                wo_s[:, a, oc * ocw:(oc + 1) * ocw],
                                start=(a == 0), stop=(a == cfg.AT - 1))
                        ys = py.tile([128, ocw], F32, tag="ys")
                        nc.vector.tensor_copy(ys[:], yps[:])
                        nc.sync.dma_start(
                            y_d[tt * 128:(tt + 1) * 128,
                                oc * ocw:(oc + 1) * ocw], ys[:])

    nc.compile()
    return nc


def host_prep(cfg, x, freq_cis, wq, wk, wv, wo):
    """Build per-core input maps (sharding + weight layout prep)."""
    HD, S = cfg.HD, cfg.S
    perm = np.concatenate([np.arange(0, HD, 2), np.arange(1, HD, 2)])
    fc = np.asarray(freq_cis, np.float32)
    A = fc[:, :, 0, 0].T
    Bc = fc[:, :, 0, 1].T
    C = fc[:, :, 1, 0].T
    Dd = fc[:, :, 1, 1].T
    p1 = np.ascontiguousarray(np.concatenate([A, Dd], 0))
    p2 = np.ascontiguousarray(np.concatenate([Bc, C], 0))
    i_idx = np.arange(128)[:, None]
    j_idx = np.arange(cfg.NQ)[None, :]
    masks = np.stack([(j_idx >= i_idx + 128 * di).astype(np.float32)
                      for di in range(cfg.NMASK)], axis=1)
    masks = np.ascontiguousarray(masks)          # [128, NMASK, NQ]

    in_maps = []
    for c in range(cfg.NCORES):
        b, tp = divmod(c, cfg.TP)
        qsl = slice(tp * cfg.QCOLS, (tp + 1) * cfg.QCOLS)
        ksl = slice(tp * cfg.KCOLS, (tp + 1) * cfg.KCOLS)
        wq_l = wq[qsl].reshape(cfg.HL, HD, cfg.D)[:, perm, :].reshape(
            cfg.QCOLS, cfg.D)
        wk_l = wk[ksl].reshape(cfg.KVL, HD, cfg.D)[:, perm, :].reshape(
            cfg.KCOLS, cfg.D)
        wv_l = wv[ksl]
        wqkvT = np.ascontiguousarray(
            np.concatenate([wq_l, wk_l, wv_l], 0).T)
        osl = slice(tp * cfg.OUTC, (tp + 1) * cfg.OUTC)
        woT = np.ascontiguousarray(wo[osl, :].T)
        in_maps.append({
            "x": np.ascontiguousarray(x[b]),
            "wqkvT": wqkvT, "woT": woT,
            "p1": p1, "p2": p2, "masks": masks,
            "ones1": np.ones((128, 1), np.float32),
            "ones2": np.ones((1, 128), np.float32),
        })
    return in_maps


def assemble(cfg, results):
    outs = []
    for b in range(cfg.B):
        parts = [results[b * cfg.TP + tp]["y"] for tp in range(cfg.TP)]
        outs.append(np.concatenate(parts, axis=1))
    return np.stack(outs, 0).astype(np.float32)


_CACHE = {}


def kernel(x, freq_cis, wq, wk, wv, wo):
    x = np.asarray(x, np.float32)
    freq_cis = np.asarray(freq_cis, np.float32)
    wq = np.asarray(wq, np.float32)
    wk = np.asarray(wk, np.float32)
    wv = np.asarray(wv, np.float32)
    wo = np.asarray(wo, np.float32)

    cfg = Cfg()
    if "nc" not in _CACHE:
        rg = [[g * cfg.TP + i for i in range(cfg.TP)] for g in range(cfg.DP)]
        _CACHE["nc"] = build_program(cfg, cfg.NCORES, rg)
    nc = _CACHE["nc"]
    in_maps = host_prep(cfg, x, freq_cis, wq, wk, wv, wo)
    res = run_bass_kernel_spmd(nc, in_maps, core_ids=list(range(cfg.NCORES)))
    return assemble(cfg, res.results)


if __name__ == "__main__":
    import reference
    inputs = {k: np.asarray(v) for k, v in reference.setup_inputs().items()}
    out = kernel(**inputs)
    exp = np.asarray(reference.reference(**inputs))
    err = np.abs(out - exp)
    denom = np.sqrt(np.mean(exp ** 2))
    print("max abs err:", err.max())
    print("rel err (rms):", np.sqrt(np.mean(err ** 2)) / denom)
